# revision 1
# baseline (speedup 1.0000x reference)
"""Causal self-attention (B=4, T=2048, C=1024, H=16) on 8 NeuronCores, v2.

Sharding: batch x head-group. Core c handles batch b = c//2 and head group
j = c%2 (8 of 16 heads). Per core: q/k in feature-major layout (qkT), v in
token-major layout (with a ones column per head for softmax sums),
flash-style causal attention with no-max softmax.

v2 changes vs v1:
- AV uses pT blocks as the STATIONARY operand (fresh ldweights are free on
  trn2) with v as the 65-col moving operand: yq[q,(d,sum)] psum accumulates
  per 128-token q block. ~8x fewer PE columns streamed for AV.
- Softmax normalization is per-partition (q on partitions): reciprocal of
  the sums column + tensor_scalar_mul. Kills the sums-broadcast matmuls.
- y (token-major) is PE-transposed (pair-packed [128,128] blocks) into
  feature-major yT for the projection.
- Output projection is split across the pair: each core computes its 512
  output features only (wp/bp/outT halved); host assembles.
All matmuls bf16 with fp32 PSUM accumulation.
"""
import numpy as np
import ml_dtypes

B, T, C, H, D = 4, 2048, 1024, 16, 64
N_CORES = 8
_BF = ml_dtypes.bfloat16

_STATE = {}


def _build_bass(loop_n=None, dbg=False, ablate=None, cc_n=1):
    import concourse.bacc as bacc
    import concourse.bass as bass
    import concourse.tile as tile
    from concourse import mybir

    BF16 = mybir.dt.bfloat16
    F32 = mybir.dt.float32

    nc = bacc.Bacc("TRN2", target_bir_lowering=False, debug=False,
                   num_devices=N_CORES)

    if dbg:
        d_myyT = nc.dram_tensor("d_myyT", [128, 4, 4, 512], BF16,
                                kind="ExternalOutput")
        d_ytf = nc.dram_tensor("d_ytf", [128, 4, 8, 512], BF16,
                               kind="ExternalOutput")

    xT = nc.dram_tensor("xT", [C, T], BF16, kind="ExternalInput")
    wqk = nc.dram_tensor("wqk", [C, 1024], BF16, kind="ExternalInput")
    wv = nc.dram_tensor("wv", [C, 512], BF16, kind="ExternalInput")
    wp = nc.dram_tensor("wp", [C, 512], BF16, kind="ExternalInput")
    bqk = nc.dram_tensor("bqk", [1024], F32, kind="ExternalInput")
    bv = nc.dram_tensor("bv", [512], F32, kind="ExternalInput")
    bp = nc.dram_tensor("bp", [512], F32, kind="ExternalInput")
    tri = nc.dram_tensor("tri", [128, 128], BF16, kind="ExternalInput")
    eye = nc.dram_tensor("eye", [128, 128], BF16, kind="ExternalInput")
    outT = nc.dram_tensor("outT", [512, T], F32, kind="ExternalOutput")

    with tile.TileContext(nc) as tc:
        with (
            tc.tile_pool(name="consts", bufs=1) as cw,
            tc.tile_pool(name="pt", bufs=4) as pc,
            tc.tile_pool(name="evac", bufs=3) as ev,
            tc.tile_pool(name="small", bufs=8) as sm,
            tc.tile_pool(name="ypair", bufs=2) as yp,
            tc.tile_pool(name="myyt", bufs=2) as my,
            tc.tile_pool(name="ppmm", bufs=2, space="PSUM") as ppmm,
            tc.tile_pool(name="pps", bufs=2, space="PSUM") as pps,
            tc.tile_pool(name="ppy", bufs=2, space="PSUM") as ppy,
            tc.tile_pool(name="dram", bufs=1, space="DRAM") as dram,
        ):
            # ---- persistent SBUF tiles ----
            xT_sb = cw.tile([128, 8, T], BF16, tag="big")
            wqk_sb = cw.tile([128, 8, 1024], BF16)
            wv_sb = cw.tile([128, 8, 512], BF16)
            wp_sb = cw.tile([128, 8, 512], BF16)
            bqk_sb = cw.tile([128, 8], F32)
            bp_sb = cw.tile([128, 4], F32)
            bv_sb = cw.tile([128, 512], F32)
            tri_sb = cw.tile([128, 128], BF16)
            eye_sb = cw.tile([128, 128], BF16)
            qkT_sb = cw.tile([128, 8, T], BF16)
            vaug_sb = cw.tile([128, 16, 8 * 65], BF16)

            # ---- input DMAs (split for queue parallelism) ----
            xT_r = xT.ap().rearrange("(a p) t -> p a t", p=128)
            wqk_r = wqk.ap().rearrange("(a p) f -> p a f", p=128)
            wv_r = wv.ap().rearrange("(a p) f -> p a f", p=128)
            wp_r = wp.ap().rearrange("(a p) f -> p a f", p=128)
            for a in range(8):
                nc.sync.dma_start(out=wqk_sb[:, a, :], in_=wqk_r[:, a, :])
                nc.sync.dma_start(out=xT_sb[:, a, :], in_=xT_r[:, a, :])
                nc.sync.dma_start(out=wv_sb[:, a, :], in_=wv_r[:, a, :])
            for a in range(8):
                nc.sync.dma_start(out=wp_sb[:, a, :], in_=wp_r[:, a, :])
            nc.sync.dma_start(out=bqk_sb[:],
                              in_=bqk.ap().rearrange("(a p) -> p a", p=128))
            nc.sync.dma_start(out=bp_sb[:],
                              in_=bp.ap().rearrange("(a p) -> p a", p=128))
            bv_bcast = bass.AP(tensor=bv.ap().tensor, offset=0,
                               ap=[[0, 128], [1, 512]])
            nc.sync.dma_start(out=bv_sb[:], in_=bv_bcast)
            nc.sync.dma_start(out=tri_sb[:], in_=tri.ap())
            nc.sync.dma_start(out=eye_sb[:], in_=eye.ap())

            def emit_body(collective=True):
                vaug4 = vaug_sb[:].rearrange("p b (h e) -> p b h e", e=65)
                nc.vector.memset(vaug4[:, :, :, 64:65], 1.0)
                bv_r = bv_sb[:].rearrange("p (h e) -> p h e", e=64)

                def qkT_tile(ts, fb):
                    # qkT[f-block, ts chunk] = sum_c wqk[c, f] xT[c, t] + bqk
                    def go():
                        ps = ppmm.tile([128, 512], F32, tag="ps")
                        for kc in range(8):
                            nc.tensor.matmul(
                                ps[:],
                                wqk_sb[:, kc, fb * 128:(fb + 1) * 128],
                                xT_sb[:, kc, ts * 512:(ts + 1) * 512],
                                start=(kc == 0), stop=(kc == 7),
                            )
                        nc.vector.tensor_scalar_add(
                            out=qkT_sb[:, fb, ts * 512:(ts + 1) * 512],
                            in0=ps[:],
                            scalar1=bqk_sb[:, fb:fb + 1],
                        )
                    return go

                def qkT_tiles(ts):
                    return [qkT_tile(ts, fb) for fb in range(8)]

                def v_tile(tb):
                    # v[t-block, f] (token-major); ones col is pre-memset
                    def go():
                        ps = ppmm.tile([128, 512], F32, tag="ps")
                        for kc in range(8):
                            nc.tensor.matmul(
                                ps[:],
                                xT_sb[:, kc, tb * 128:(tb + 1) * 128],
                                wv_sb[:, kc, :],
                                start=(kc == 0), stop=(kc == 7),
                            )
                        nc.vector.tensor_add(
                            out=vaug4[:, tb, :, 0:64],
                            in0=ps[:].rearrange("p (h e) -> p h e", e=64),
                            in1=bv_r,
                        )
                    return go

                cc_in_q = []
                cc_out_q = []
                for tsq in range(4):
                    ci = dram.tile([512, 512], BF16, name=f"cc_in_{tsq}")
                    co = dram.tile([1024, 512], BF16, name=f"cc_out_{tsq}")
                    cc_in_q.append(ci)
                    cc_out_q.append(co)

                def proj_tile(yTf_c, q0, mb):
                    # outT rows mb*128..: my half of the output features
                    def go():
                        ps = ppmm.tile([128, 512], F32, tag="ps")
                        for kc in range(8):
                            nc.tensor.matmul(
                                ps[:],
                                wp_sb[:, kc, mb * 128:(mb + 1) * 128],
                                yTf_c[:, kc, :],
                                start=(kc == 0), stop=(kc == 7),
                            )
                        o_sb = ev.tile([128, 512], F32)
                        nc.vector.tensor_scalar_add(
                            out=o_sb[:], in0=ps[:], scalar1=bp_sb[:, mb:mb + 1])
                        nc.sync.dma_start(
                            out=outT.ap()[mb * 128:(mb + 1) * 128, q0:q0 + 512],
                            in_=o_sb[:],
                        )
                    return go

                def proj_tiles(yTf_c, q0):
                    return [proj_tile(yTf_c, q0, mb) for mb in range(4)]

                from collections import deque
                fillers = deque()

                def drain(n):
                    for _ in range(min(n, len(fillers))):
                        fillers.popleft()()

                # prologue: only what chunk 0 / head-pair 0 needs up
                # front (q tile fb0, k tile fb4, v blocks 0-3); remaining
                # chunk-0 qkT tiles ride as fillers in hp0-hp2 exp bubbles
                qkT_tile(0, 0)()
                qkT_tile(0, 4)()
                for tb in range(4):
                    v_tile(tb)()
                for fb in (1, 5, 2, 6, 3, 7):
                    fillers.append(qkT_tile(0, fb))

                pending_proj = None
                for tsq in range(4):
                    q0 = tsq * 512
                    nkb = 4 * (tsq + 1)
                    ngrp = nkb // 2
                    if tsq < 3:
                        fillers.extend(qkT_tiles(tsq + 1))
                        for tb in range(4 * tsq + 4, 4 * tsq + 8):
                            fillers.append(v_tile(tb))
                    if pending_proj is not None:
                        fillers.extend(proj_tiles(*pending_proj))
                        pending_proj = None

                    # ---- pairwise AllGather buffers for this T-chunk ----
                    ci, co = cc_in_q[tsq], cc_out_q[tsq]
                    ci_r = ci[:].rearrange("(a p) t -> p a t", p=128)
                    my_yT = my.tile([128, 4, 512], BF16, tag="myyt")
                    if ablate in ("noav", "noattn"):
                        nc.vector.memset(my_yT[:], 0.125)
                    pending_av = None
                    for hp in ([] if ablate == "noattn" else range(4)):
                        h0, h1 = 2 * hp, 2 * hp + 1
                        fq = hp
                        fk = 4 + hp
                        pts = []
                        for g in range(ngrp):
                            diag = g >= 2 * tsq
                            spss = [pps.tile([128, 1024], F32, tag="sps",
                                             name=f"sps{h}")
                                    for h in (h0, h1)]
                            pTs = [pc.tile([128, 1024], BF16, tag="pT",
                                           name=f"pT{h}", bufs=18)
                                   for h in (h0, h1)]
                            pts.append(pTs)
                            # scores: adjacent emission of the two heads'
                            # matmuls -> disjoint PE row groups (partition
                            # bases 0 and 64) run concurrently
                            for u in range(2):
                                kb = 2 * g + u
                                r = kb - 4 * tsq
                                for hi, h in enumerate((h0, h1)):
                                    po = (h % 2) * 64
                                    if r < 0:
                                        nc.tensor.matmul(
                                            spss[hi][:, u * 512:(u + 1) * 512],
                                            qkT_sb[po:po + 64, fk,
                                                   kb * 128:(kb + 1) * 128],
                                            qkT_sb[po:po + 64, fq,
                                                   q0:q0 + 512],
                                            start=True, stop=True,
                                        )
                                    else:
                                        nc.tensor.matmul(
                                            spss[hi][:, u * 512 + r * 128:
                                                     (u + 1) * 512],
                                            qkT_sb[po:po + 64, fk,
                                                   kb * 128:(kb + 1) * 128],
                                            qkT_sb[po:po + 64, fq,
                                                   q0 + r * 128:q0 + 512],
                                            start=True, stop=True,
                                        )
                            # exp (+ causal mask on diagonal blocks)
                            for hi in range(2):
                                if not diag:
                                    nc.scalar.activation(
                                        out=pTs[hi][:], in_=spss[hi][:],
                                        func=mybir.ActivationFunctionType.Exp,
                                        scale=0.125,
                                    )
                                else:
                                    for u in range(2):
                                        kb = 2 * g + u
                                        r = kb - 4 * tsq
                                        sl = slice(u * 512 + r * 128,
                                                   (u + 1) * 512)
                                        nc.scalar.activation(
                                            out=pTs[hi][:, sl],
                                            in_=spss[hi][:, sl],
                                            func=mybir.ActivationFunctionType.Exp,
                                            scale=0.125,
                                        )
                                        dsl = slice(u * 512 + r * 128,
                                                    u * 512 + r * 128 + 128)
                                        nc.vector.tensor_mul(
                                            out=pTs[hi][:, dsl],
                                            in0=pTs[hi][:, dsl],
                                            in1=tri_sb[:],
                                        )
                            # filler matmuls ride in the PE bubble while ACT
                            # computes the exps
                            drain(2)
                            if g == 0 and pending_av is not None:
                                pending_av()
                                pending_av = None
                        if ablate == "noav":
                            continue
                        # AV sweep: pT block stationary, v (64 cols + ones
                        # col) moving; one complete kb sweep per (head, qb)
                        # psum tile. A tile's accumulation group must fully
                        # close before another opens in the same tile, and
                        # concurrently-open groups must live in separate
                        # tiles (see mb3.py) - hence qb-outer sweeps and
                        # 4 rotating tile tags.
                        # normalize: y /= sums col (per-partition scalars),
                        # pack the pair into [128 q, 4 qb, 128 d2]
                        y_pair = yp.tile([128, 4, 128], BF16, tag="ypair")
                        yqs = [ppy.tile([128, 4, 65], F32, tag=f"yq{hi}",
                                        bufs=1, name=f"yq{h}")
                               for hi, h in enumerate((h0, h1))]

                        def av_sweeps(qbs, yqs=yqs, y_pair=y_pair, tsq=tsq,
                                      pts=pts, heads=(h0, h1)):
                            for qb in qbs:
                                for hi, h in enumerate(heads):
                                    yq = yqs[hi][:, qb, :]
                                    for kb in range(4 * tsq + qb + 1):
                                        g, u = kb // 2, kb % 2
                                        nc.tensor.matmul(
                                            yq,
                                            pts[g][hi][
                                                :, u * 512 + qb * 128:
                                                u * 512 + (qb + 1) * 128],
                                            vaug4[:, kb, h, :],
                                            start=(kb == 0),
                                            stop=(kb == 4 * tsq + qb),
                                            skip_group_check=True,
                                        )
                            if ablate == "sweeponly" or qbs[0] != 2:
                                return
                            # batched normalize for all 4 qb of each head:
                            # one strided reciprocal of the 4 sums columns,
                            # one stride-0-broadcast tensor_mul
                            for hi, h in enumerate(heads):
                                recip4 = sm.tile([128, 4], F32, tag="recip",
                                                 name=f"re{hi}")
                                nc.vector.reciprocal_approx_fast(
                                    out=recip4[:],
                                    in_=yqs[hi][:, :, 64:65])
                                r_ap = recip4[:]
                                r_b = bass.AP(
                                    tensor=r_ap.tensor, offset=r_ap.offset,
                                    ap=list(r_ap.ap[:-1]) + [list(r_ap.ap[-1]),
                                                             [0, 64]])
                                nc.vector.tensor_mul(
                                    out=y_pair[:, :, hi * 64:(hi + 1) * 64],
                                    in0=yqs[hi][:, :, 0:64],
                                    in1=r_b,
                                )

                        # qb 0/1 need only groups <= ngrp-2: they run on PE
                        # during the last group's exp
                        av_sweeps((0, 1))

                        def av_part2(hp=hp, av_sweeps=av_sweeps,
                                     y_pair=y_pair, my_yT=my_yT):
                            av_sweeps((2, 3))
                            if ablate == "sweeponly":
                                nc.vector.memset(my_yT[:, hp, :], 0.125)
                                return
                            drain(1)
                            # transpose pair blocks -> feature-major yT
                            ypsT = ppmm.tile([128, 4, 128], BF16, tag="ps",
                                             name=f"ypsT{hp}")
                            for qb in range(4):
                                nc.tensor.transpose(
                                    ypsT[:, qb, :], y_pair[:, qb, :],
                                    eye_sb[:])
                            nc.vector.tensor_copy(
                                my_yT[:, hp, :],
                                ypsT[:].rearrange("p a q -> p (a q)"))
                            nc.sync.dma_start(out=ci_r[:, hp, :],
                                              in_=my_yT[:, hp, :])
                            drain(1)
                        pending_av = av_part2

                    if pending_av is not None:
                        pending_av()
                        pending_av = None
                    if ablate in ("noav", "noattn", "sweeponly"):
                        nc.sync.dma_start(out=ci_r[:], in_=my_yT[:])
                    if collective:
                        cur_in = ci.opt()
                        for icc in range(cc_n - 1):
                            co_x = dram.tile([1024, 512], BF16,
                                             name=f"cc_x_{tsq}_{icc}")
                            nc.gpsimd.collective_compute(
                                "AllGather",
                                mybir.AluOpType.bypass,
                                replica_groups=[[0, 1], [2, 3], [4, 5],
                                                [6, 7]],
                                ins=[cur_in],
                                outs=[co_x.opt()],
                            )
                            cur_in = co_x[0:512, :]
                        nc.gpsimd.collective_compute(
                            "AllGather",
                            mybir.AluOpType.bypass,
                            replica_groups=[[0, 1], [2, 3], [4, 5], [6, 7]],
                            ins=[cur_in],
                            outs=[co.opt()],
                        )
                    yTf_c = cw.tile([128, 8, 512], BF16, tag="ytf", bufs=2)
                    co_r = co[:].rearrange("(a p) t -> p a t", p=128)
                    ci_rb = ci[:].rearrange("(a p) t -> p a t", p=128)
                    for a in range(8):
                        if collective:
                            nc.sync.dma_start(out=yTf_c[:, a, :],
                                              in_=co_r[:, a, :])
                        else:
                            nc.sync.dma_start(out=yTf_c[:, a, :],
                                              in_=ci_rb[:, a % 4, :])
                    if dbg:
                        nc.sync.dma_start(out=d_myyT.ap()[:, tsq, :, :],
                                          in_=my_yT[:])
                        for a in range(8):
                            nc.sync.dma_start(
                                out=d_ytf.ap()[:, tsq, a, :],
                                in_=yTf_c[:, a, :])
                    drain(len(fillers))
                    pending_proj = (yTf_c, q0)
                for f in proj_tiles(*pending_proj):
                    f()
                return cc_out_q

            if loop_n is None:
                emit_body(collective=True)
            else:
                with tc.For_i(0, loop_n, 1) as _i:
                    emit_body(collective=False)

    nc.compile()
    return nc


def _prep_core(x, W_attn, b_attn, W_proj, b_proj, c):
    b, j = c // 2, c % 2
    xT = np.ascontiguousarray(x[b].T).astype(_BF)
    wq = W_attn[:, j * 512:(j + 1) * 512]
    wk = W_attn[:, 1024 + j * 512:1024 + (j + 1) * 512]
    wv = W_attn[:, 2048 + j * 512:2048 + (j + 1) * 512]
    return {
        "xT": xT,
        "wqk": np.concatenate([wq, wk], axis=1).astype(_BF),
        "wv": np.ascontiguousarray(wv).astype(_BF),
        "wp": np.ascontiguousarray(
            W_proj[:, j * 512:(j + 1) * 512]).astype(_BF),
        "bqk": np.concatenate([b_attn[j * 512:(j + 1) * 512],
                               b_attn[1024 + j * 512:1024 + (j + 1) * 512]]
                              ).astype(np.float32),
        "bv": np.ascontiguousarray(b_attn[2048 + j * 512:2048 + (j + 1) * 512]
                                   ).astype(np.float32),
        "bp": np.ascontiguousarray(
            b_proj[j * 512:(j + 1) * 512]).astype(np.float32),
        "tri": np.tril(np.ones((128, 128), np.float32)).T.astype(_BF),
        "eye": np.eye(128, dtype=np.float32).astype(_BF),
    }


def kernel(x, W_attn, b_attn, W_proj, b_proj):
    from concourse import bass_utils

    x = np.asarray(x, dtype=np.float32)
    W_attn = np.asarray(W_attn, dtype=np.float32)
    b_attn = np.asarray(b_attn, dtype=np.float32)
    W_proj = np.asarray(W_proj, dtype=np.float32)
    b_proj = np.asarray(b_proj, dtype=np.float32)

    if "nc" not in _STATE:
        _STATE["nc"] = _build_bass()
    nc = _STATE["nc"]

    in_maps = [_prep_core(x, W_attn, b_attn, W_proj, b_proj, c)
               for c in range(N_CORES)]
    # the axon terminal occasionally dies with a transient
    # "worker hung up" / NRT_EXEC_UNIT_UNRECOVERABLE — retry
    last_exc = None
    for attempt in range(3):
        try:
            res = bass_utils.run_bass_kernel_spmd(
                nc, in_maps, core_ids=list(range(N_CORES)))
            break
        except Exception as e:  # noqa: BLE001
            last_exc = e
            import time
            time.sleep(10 * (attempt + 1))
    else:
        raise last_exc

    out = np.empty((B, T, C), dtype=np.float32)
    for b in range(B):
        full = np.concatenate(
            [res.results[2 * b]["outT"], res.results[2 * b + 1]["outT"]],
            axis=0)
        out[b] = full.T
    return out



# revision 3
# speedup vs baseline: 1.1357x; 1.1357x over previous
"""Causal self-attention (B=4, T=2048, C=1024, H=16) on 8 NeuronCores, v4.

Sharding: batch x head-group, as v2 (core c: batch c//2, head-group c%2).
Math identical to v2 (qkT feature-major, flash-style no-max softmax with a
ones column for sums, pT-stationary AV, PE-transpose to yT, pairwise
AllGather, half-feature output projection).

v4 changes vs v2 (scheduling; found via concourse TimelineSim attribution,
which matches HW within a few percent):
- PE is the bottleneck engine (~201us busy of ~250us makespan); the ACT
  exp chain (~156us) is the secondary serial floor. v2 starved ACT ~60us:
  AV sweeps + av_part2 + fixed drain(2) sat between consecutive groups'
  score matmuls in the PE stream.
- Every non-score PE task is now a cost-tagged unit in priority queues
  (hard: next pass's qkT tiles; av: v tiles + AV sweeps + finalize; soft:
  proj tiles). After each group's exp, drain() pops ~one exp-duration of
  PE work (per-chunk budget BUD), so the next scores always land before
  ACT finishes the previous exp.
- AV sweeps are per-(qb, head) units, drain-eligible 2 groups after their
  last pT dependency (popping one never blocks the PE FIFO on a running
  exp). v2's units were too big to fit the drain budget at chunk 3 and
  all dumped serially at chunk end.
- proj units are gated out of drains until chunk 3, filling the
  ACT-bound tail region where qkT/v filler supply is exhausted.
- r=0 diagonal groups exp the full [128,1024] tile in one ACT call (the
  overhead of a second trimmed call exceeds the 128 garbage cols, which
  AV never reads).
- Input DMAs ordered by first use (consts, wqk fb0/fb4 slices + xT ts0,
  then wv/rest) so the first qkT tiles start ~6us in, not ~23us.
- pT pool bufs 18 -> 24 (AV deferral extends pT lifetime ~one pass),
  yTf bufs 2 -> 3 (proj deferral extends gather-tile lifetime).
"""
import numpy as np
import ml_dtypes

B, T, C, H, D = 4, 2048, 1024, 16, 64
N_CORES = 8
_BF = ml_dtypes.bfloat16

_STATE = {}


def _build_bass(loop_n=None, ablate=None, cc_n=1, collective=True):
    import concourse.bacc as bacc
    import concourse.bass as bass
    import concourse.tile as tile
    from concourse import mybir
    from collections import deque

    BF16 = mybir.dt.bfloat16
    F32 = mybir.dt.float32

    nc = bacc.Bacc("TRN2", target_bir_lowering=False, debug=False,
                   num_devices=N_CORES)

    xT = nc.dram_tensor("xT", [C, T], BF16, kind="ExternalInput")
    wqk = nc.dram_tensor("wqk", [C, 1024], BF16, kind="ExternalInput")
    wv = nc.dram_tensor("wv", [C, 512], BF16, kind="ExternalInput")
    wp = nc.dram_tensor("wp", [C, 512], BF16, kind="ExternalInput")
    bqk = nc.dram_tensor("bqk", [1024], F32, kind="ExternalInput")
    bv = nc.dram_tensor("bv", [512], F32, kind="ExternalInput")
    bp = nc.dram_tensor("bp", [512], F32, kind="ExternalInput")
    tri = nc.dram_tensor("tri", [128, 128], BF16, kind="ExternalInput")
    eye = nc.dram_tensor("eye", [128, 128], BF16, kind="ExternalInput")
    outT = nc.dram_tensor("outT", [512, T], F32, kind="ExternalOutput")

    with tile.TileContext(nc) as tc:
        with (
            tc.tile_pool(name="consts", bufs=1) as cw,
            tc.tile_pool(name="pt", bufs=4) as pc,
            tc.tile_pool(name="evac", bufs=3) as ev,
            tc.tile_pool(name="small", bufs=8) as sm,
            tc.tile_pool(name="ypair", bufs=2) as yp,
            tc.tile_pool(name="myyt", bufs=2) as my,
            tc.tile_pool(name="ppmm", bufs=2, space="PSUM") as ppmm,
            tc.tile_pool(name="pps", bufs=2, space="PSUM") as pps,
            tc.tile_pool(name="ppy", bufs=2, space="PSUM") as ppy,
            tc.tile_pool(name="dram", bufs=1, space="DRAM") as dram,
        ):
            # ---- persistent SBUF tiles ----
            xT_sb = cw.tile([128, 8, T], BF16, tag="big")
            wqk_sb = cw.tile([128, 8, 1024], BF16)
            wv_sb = cw.tile([128, 8, 512], BF16)
            wp_sb = cw.tile([128, 8, 512], BF16)
            bqk_sb = cw.tile([128, 8], F32)
            bp_sb = cw.tile([128, 4], F32)
            bv_sb = cw.tile([128, 512], F32)
            tri_sb = cw.tile([128, 128], BF16)
            eye_sb = cw.tile([128, 128], BF16)
            qkT_sb = cw.tile([128, 8, T], BF16)
            vaug_sb = cw.tile([128, 16, 8 * 65], BF16)

            # ---- input DMAs, ordered by first use: small consts, then the
            # slices pass (0,0) needs (wqk fb0/fb4 cols + xT ts0), then rest
            xT_r = xT.ap().rearrange("(a p) t -> p a t", p=128)
            wqk_r = wqk.ap().rearrange("(a p) f -> p a f", p=128)
            wv_r = wv.ap().rearrange("(a p) f -> p a f", p=128)
            wp_r = wp.ap().rearrange("(a p) f -> p a f", p=128)
            nc.sync.dma_start(out=bqk_sb[:],
                              in_=bqk.ap().rearrange("(a p) -> p a", p=128))
            nc.sync.dma_start(out=tri_sb[:], in_=tri.ap())
            nc.sync.dma_start(out=eye_sb[:], in_=eye.ap())
            nc.sync.dma_start(out=bp_sb[:],
                              in_=bp.ap().rearrange("(a p) -> p a", p=128))
            bv_bcast = bass.AP(tensor=bv.ap().tensor, offset=0,
                               ap=[[0, 128], [1, 512]])
            nc.sync.dma_start(out=bv_sb[:], in_=bv_bcast)
            for a in range(8):
                nc.sync.dma_start(out=wqk_sb[:, a, 0:128],
                                  in_=wqk_r[:, a, 0:128])
                nc.sync.dma_start(out=wqk_sb[:, a, 512:640],
                                  in_=wqk_r[:, a, 512:640])
                nc.sync.dma_start(out=xT_sb[:, a, 0:512],
                                  in_=xT_r[:, a, 0:512])
            for a in range(8):
                nc.sync.dma_start(out=wv_sb[:, a, :], in_=wv_r[:, a, :])
                nc.sync.dma_start(out=wqk_sb[:, a, 128:512],
                                  in_=wqk_r[:, a, 128:512])
                nc.sync.dma_start(out=wqk_sb[:, a, 640:1024],
                                  in_=wqk_r[:, a, 640:1024])
            for a in range(8):
                nc.sync.dma_start(out=xT_sb[:, a, 512:2048],
                                  in_=xT_r[:, a, 512:2048])
                nc.sync.dma_start(out=wp_sb[:, a, :], in_=wp_r[:, a, :])

            def emit_body(collective=True):
                vaug4 = vaug_sb[:].rearrange("p b (h e) -> p b h e", e=65)
                nc.vector.memset(vaug4[:, :, :, 64:65], 1.0)
                bv_r = bv_sb[:].rearrange("p (h e) -> p h e", e=64)

                def qkT_tile(ts, fb):
                    def go():
                        ps = ppmm.tile([128, 512], F32, tag="ps")
                        for kc in range(8):
                            nc.tensor.matmul(
                                ps[:],
                                wqk_sb[:, kc, fb * 128:(fb + 1) * 128],
                                xT_sb[:, kc, ts * 512:(ts + 1) * 512],
                                start=(kc == 0), stop=(kc == 7),
                            )
                        nc.vector.tensor_scalar_add(
                            out=qkT_sb[:, fb, ts * 512:(ts + 1) * 512],
                            in0=ps[:],
                            scalar1=bqk_sb[:, fb:fb + 1],
                        )
                    return go

                def v_tile(tb):
                    def go():
                        ps = ppmm.tile([128, 512], F32, tag="ps")
                        for kc in range(8):
                            nc.tensor.matmul(
                                ps[:],
                                xT_sb[:, kc, tb * 128:(tb + 1) * 128],
                                wv_sb[:, kc, :],
                                start=(kc == 0), stop=(kc == 7),
                            )
                        nc.vector.tensor_add(
                            out=vaug4[:, tb, :, 0:64],
                            in0=ps[:].rearrange("p (h e) -> p h e", e=64),
                            in1=bv_r,
                        )
                    return go

                cc_in_q = []
                cc_out_q = []
                for tsq in range(4):
                    ci = dram.tile([512, 512], BF16, name=f"cc_in_{tsq}")
                    co = dram.tile([1024, 512], BF16, name=f"cc_out_{tsq}")
                    cc_in_q.append(ci)
                    cc_out_q.append(co)

                def proj_tile(yTf_c, q0, mb):
                    def go():
                        ps = ppmm.tile([128, 512], F32, tag="ps")
                        for kc in range(8):
                            nc.tensor.matmul(
                                ps[:],
                                wp_sb[:, kc, mb * 128:(mb + 1) * 128],
                                yTf_c[:, kc, :],
                                start=(kc == 0), stop=(kc == 7),
                            )
                        o_sb = ev.tile([128, 512], F32)
                        nc.vector.tensor_scalar_add(
                            out=o_sb[:], in0=ps[:], scalar1=bp_sb[:, mb:mb + 1])
                        nc.sync.dma_start(
                            out=outT.ap()[mb * 128:(mb + 1) * 128, q0:q0 + 512],
                            in_=o_sb[:],
                        )
                    return go

                # ---- priority work queues of (cost_ns, emit_fn) ----
                q_hard = deque()   # qkT tiles of the next pass
                q_av = deque()     # v tiles + AV sweeps + finalize, FIFO
                q_soft = deque()   # proj tiles

                def drain(budget, allow_soft=True):
                    spent = 0
                    while True:
                        if q_hard:
                            q = q_hard
                        elif q_av:
                            q = q_av
                        elif q_soft and allow_soft:
                            q = q_soft
                        else:
                            break
                        cost, fn = q[0]
                        if spent + cost > budget:
                            break
                        q.popleft()
                        fn()
                        spent += cost

                def flush(q):
                    while q:
                        q.popleft()[1]()

                # per-chunk drain budget (ns of PE work per group slot),
                # tuned via TimelineSim sweep: exp per group is ~2.3us on
                # ACT; scores take ~0.43us; budget fills the rest. Chunk 0
                # is PE-bound (bigger budget OK); chunk 3 is ACT-bound and
                # tolerates 2200 (2600+ delays the exp chain).
                BUD = (3400, 1900, 1800, 2200)
                C_TILE = 1700

                # prologue: just what pass (0,0) needs
                qkT_tile(0, 0)()
                qkT_tile(0, 4)()

                passes = [(tsq, hp) for tsq in range(4) for hp in range(4)]
                my_yT_cur = [None]  # chunk-scoped [128, 4, 512] tile

                def make_av_units(tsq, hp, pts, ci_r, heads):
                    cell = {}
                    my_yT = my_yT_cur[0]

                    def alloc():
                        cell["yqs"] = [
                            ppy.tile([128, 4, 65], F32, tag=f"yq{hi}",
                                     bufs=1, name=f"yq{h}")
                            for hi, h in enumerate(heads)]
                        cell["y_pair"] = yp.tile([128, 4, 128], BF16,
                                                 tag="ypair",
                                                 name=f"y_pair_{tsq}_{hp}")

                    def sweep(qb, hi):
                        def go():
                            if qb == 0 and hi == 0:
                                alloc()
                            yq = cell["yqs"][hi][:, qb, :]
                            h = heads[hi]
                            for kb in range(4 * tsq + qb + 1):
                                g, u = kb // 2, kb % 2
                                nc.tensor.matmul(
                                    yq,
                                    pts[g][hi][
                                        :, u * 512 + qb * 128:
                                        u * 512 + (qb + 1) * 128],
                                    vaug4[:, kb, h, :],
                                    start=(kb == 0),
                                    stop=(kb == 4 * tsq + qb),
                                    skip_group_check=True,
                                )
                        return go

                    def normalize():
                        yqs = cell["yqs"]
                        y_pair = cell["y_pair"]
                        for hi in range(2):
                            recip4 = sm.tile([128, 4], F32, tag="recip",
                                             name=f"re{hi}")
                            nc.vector.reciprocal_approx_fast(
                                out=recip4[:],
                                in_=yqs[hi][:, :, 64:65])
                            r_ap = recip4[:]
                            r_b = bass.AP(
                                tensor=r_ap.tensor, offset=r_ap.offset,
                                ap=list(r_ap.ap[:-1]) + [list(r_ap.ap[-1]),
                                                         [0, 64]])
                            nc.vector.tensor_mul(
                                out=y_pair[:, :, hi * 64:(hi + 1) * 64],
                                in0=yqs[hi][:, :, 0:64],
                                in1=r_b,
                            )

                    def fin():
                        y_pair = cell["y_pair"]
                        ypsT = ppmm.tile([128, 4, 128], BF16, tag="ps",
                                         name=f"ypsT{hp}")
                        for qb in range(4):
                            nc.tensor.transpose(
                                ypsT[:, qb, :], y_pair[:, qb, :],
                                eye_sb[:])
                        nc.vector.tensor_copy(
                            my_yT[:, hp, :],
                            ypsT[:].rearrange("p a q -> p (a q)"))
                        nc.sync.dma_start(out=ci_r[:, hp, :],
                                          in_=my_yT[:, hp, :])

                    # sweep (qb, hi) reads pT groups <= (4*tsq+qb)//2; make
                    # it drain-eligible 2 groups later so popping it never
                    # blocks the PE FIFO on a still-running exp
                    ready = {}
                    for qb in range(4):
                        for hi in range(2):
                            c = 55 * (4 * tsq + qb + 1) + 75
                            g_dep = (4 * tsq + qb) // 2
                            ready.setdefault(g_dep + 2, []).append(
                                (c, sweep(qb, hi)))
                    tail = [(100, normalize), (500, fin)]
                    return ready, tail

                for pi, (tsq, hp) in enumerate(passes):
                    q0 = tsq * 512
                    nkb = 4 * (tsq + 1)
                    ngrp = nkb // 2
                    if pi + 1 < len(passes):
                        nt, nh = passes[pi + 1]
                        q_hard.append((C_TILE, qkT_tile(nt, nh)))
                        q_hard.append((C_TILE, qkT_tile(nt, 4 + nh)))
                    if hp == 0:
                        for tb in range(4 * tsq, 4 * tsq + 4):
                            q_av.append((C_TILE, v_tile(tb)))
                        my_yT_cur[0] = my.tile([128, 4, 512], BF16,
                                               tag="myyt",
                                               name=f"my_yT_{tsq}")

                    ci, co = cc_in_q[tsq], cc_out_q[tsq]
                    ci_r = ci[:].rearrange("(a p) t -> p a t", p=128)
                    h0, h1 = 2 * hp, 2 * hp + 1
                    fq = hp
                    fk = 4 + hp
                    pts = []
                    av_ready, av_tail = make_av_units(tsq, hp, pts, ci_r,
                                                      (h0, h1))
                    for g in range(ngrp):
                        for unit in av_ready.pop(g, []):
                            q_av.append(unit)
                        diag = g >= 2 * tsq
                        spss = [pps.tile([128, 1024], F32, tag="sps",
                                         name=f"sps{h}")
                                for h in (h0, h1)]
                        pTs = [pc.tile([128, 1024], BF16, tag="pT",
                                       name=f"pT{h}", bufs=24)
                               for h in (h0, h1)]
                        pts.append(pTs)
                        for u in range(2):
                            kb = 2 * g + u
                            r = kb - 4 * tsq
                            for hi, h in enumerate((h0, h1)):
                                po = (h % 2) * 64
                                if r < 0:
                                    nc.tensor.matmul(
                                        spss[hi][:, u * 512:(u + 1) * 512],
                                        qkT_sb[po:po + 64, fk,
                                               kb * 128:(kb + 1) * 128],
                                        qkT_sb[po:po + 64, fq,
                                               q0:q0 + 512],
                                        start=True, stop=True,
                                    )
                                else:
                                    nc.tensor.matmul(
                                        spss[hi][:, u * 512 + r * 128:
                                                 (u + 1) * 512],
                                        qkT_sb[po:po + 64, fk,
                                               kb * 128:(kb + 1) * 128],
                                        qkT_sb[po:po + 64, fq,
                                               q0 + r * 128:q0 + 512],
                                        start=True, stop=True,
                                    )
                        for hi in range(2):
                            if not diag or 2 * g == 4 * tsq:
                                # r=0 diag group: one full call is cheaper
                                # than two trimmed ones (the 352-cyc call
                                # overhead beats 128 garbage cols; the
                                # garbage region is never read by AV)
                                nc.scalar.activation(
                                    out=pTs[hi][:], in_=spss[hi][:],
                                    func=mybir.ActivationFunctionType.Exp,
                                    scale=0.125,
                                )
                                if not diag:
                                    continue
                                for u in range(2):
                                    r = 2 * g + u - 4 * tsq
                                    dsl = slice(u * 512 + r * 128,
                                                u * 512 + r * 128 + 128)
                                    nc.vector.tensor_mul(
                                        out=pTs[hi][:, dsl],
                                        in0=pTs[hi][:, dsl],
                                        in1=tri_sb[:],
                                    )
                            else:
                                for u in range(2):
                                    kb = 2 * g + u
                                    r = kb - 4 * tsq
                                    sl = slice(u * 512 + r * 128,
                                               (u + 1) * 512)
                                    nc.scalar.activation(
                                        out=pTs[hi][:, sl],
                                        in_=spss[hi][:, sl],
                                        func=mybir.ActivationFunctionType.Exp,
                                        scale=0.125,
                                    )
                                    dsl = slice(u * 512 + r * 128,
                                                u * 512 + r * 128 + 128)
                                    nc.vector.tensor_mul(
                                        out=pTs[hi][:, dsl],
                                        in0=pTs[hi][:, dsl],
                                        in1=tri_sb[:],
                                    )
                        drain(BUD[tsq], allow_soft=(tsq == 3))

                    for g_key in sorted(av_ready):
                        for unit in av_ready[g_key]:
                            q_av.append(unit)
                    for unit in av_tail:
                        q_av.append(unit)

                    if hp == 3:
                        # chunk end: AV + finalize of this chunk must be
                        # emitted before the collective reads ci
                        flush(q_av)
                        if collective:
                            cur_in = ci.opt()
                            for icc in range(cc_n - 1):
                                co_x = dram.tile([1024, 512], BF16,
                                                 name=f"cc_x_{tsq}_{icc}")
                                nc.gpsimd.collective_compute(
                                    "AllGather",
                                    mybir.AluOpType.bypass,
                                    replica_groups=[[0, 1], [2, 3], [4, 5],
                                                    [6, 7]],
                                    ins=[cur_in],
                                    outs=[co_x.opt()],
                                )
                                cur_in = co_x[0:512, :]
                            nc.gpsimd.collective_compute(
                                "AllGather",
                                mybir.AluOpType.bypass,
                                replica_groups=[[0, 1], [2, 3], [4, 5],
                                                [6, 7]],
                                ins=[cur_in],
                                outs=[co.opt()],
                            )
                        yTf_c = cw.tile([128, 8, 512], BF16, tag="ytf",
                                        bufs=3)
                        co_r = co[:].rearrange("(a p) t -> p a t", p=128)
                        ci_rb = ci[:].rearrange("(a p) t -> p a t", p=128)
                        for a in range(8):
                            if collective:
                                nc.sync.dma_start(out=yTf_c[:, a, :],
                                                  in_=co_r[:, a, :])
                            else:
                                nc.sync.dma_start(out=yTf_c[:, a, :],
                                                  in_=ci_rb[:, a % 4, :])
                        for mb in range(4):
                            q_soft.append((C_TILE, proj_tile(yTf_c, q0, mb)))

                flush(q_hard)
                flush(q_av)
                flush(q_soft)
                return cc_out_q

            if loop_n is None:
                emit_body(collective=collective)
            else:
                with tc.For_i(0, loop_n, 1) as _i:
                    emit_body(collective=False)

    nc.compile()
    return nc


def _prep_core(x, W_attn, b_attn, W_proj, b_proj, c):
    b, j = c // 2, c % 2
    xT = np.ascontiguousarray(x[b].T).astype(_BF)
    wq = W_attn[:, j * 512:(j + 1) * 512]
    wk = W_attn[:, 1024 + j * 512:1024 + (j + 1) * 512]
    wv = W_attn[:, 2048 + j * 512:2048 + (j + 1) * 512]
    return {
        "xT": xT,
        "wqk": np.concatenate([wq, wk], axis=1).astype(_BF),
        "wv": np.ascontiguousarray(wv).astype(_BF),
        "wp": np.ascontiguousarray(
            W_proj[:, j * 512:(j + 1) * 512]).astype(_BF),
        "bqk": np.concatenate([b_attn[j * 512:(j + 1) * 512],
                               b_attn[1024 + j * 512:1024 + (j + 1) * 512]]
                              ).astype(np.float32),
        "bv": np.ascontiguousarray(b_attn[2048 + j * 512:2048 + (j + 1) * 512]
                                   ).astype(np.float32),
        "bp": np.ascontiguousarray(
            b_proj[j * 512:(j + 1) * 512]).astype(np.float32),
        "tri": np.tril(np.ones((128, 128), np.float32)).T.astype(_BF),
        "eye": np.eye(128, dtype=np.float32).astype(_BF),
    }


def kernel(x, W_attn, b_attn, W_proj, b_proj):
    from concourse import bass_utils

    x = np.asarray(x, dtype=np.float32)
    W_attn = np.asarray(W_attn, dtype=np.float32)
    b_attn = np.asarray(b_attn, dtype=np.float32)
    W_proj = np.asarray(W_proj, dtype=np.float32)
    b_proj = np.asarray(b_proj, dtype=np.float32)

    if "nc" not in _STATE:
        _STATE["nc"] = _build_bass()
    nc = _STATE["nc"]

    in_maps = [_prep_core(x, W_attn, b_attn, W_proj, b_proj, c)
               for c in range(N_CORES)]
    last_exc = None
    for attempt in range(3):
        try:
            res = bass_utils.run_bass_kernel_spmd(
                nc, in_maps, core_ids=list(range(N_CORES)))
            break
        except Exception as e:  # noqa: BLE001
            last_exc = e
            import time
            time.sleep(10 * (attempt + 1))
    else:
        raise last_exc

    out = np.empty((B, T, C), dtype=np.float32)
    for b in range(B):
        full = np.concatenate(
            [res.results[2 * b]["outT"], res.results[2 * b + 1]["outT"]],
            axis=0)
        out[b] = full.T
    return out
